# revision 3
# baseline (speedup 1.0000x reference)
"""TRN2 Bass kernel for nn_Attention_39316130628152 (v3).

Spatial self-attention: B=4, C=64, H=W=64 (N=4096 tokens), f32.
  q/k/v = 1x1conv(x);  out = v @ softmax(q^T k)^T

Sharding: 8 cores = (batch b in 0..3) x (query-half h in 0..1).
Each core: 2048 queries x 4096 keys for one batch.

Key ideas vs v1 (~95us):
  1. Bias algebra: softmax_j(q_i.k_j) = softmax_j(x_j.(Gm x_i) + w2.x_j)
     with Gm = Wk^T Wq, w2 = Wk^T bq (i-only terms cancel in softmax).
     The k/q projections collapse into ONE K=64 matmul g = Gm x, and the
     per-j bias term is folded into the HOST-prepared U weights:
     XT rows are scaled by exp(w2.x_j), which multiplies softmax
     numerator AND denominator identically -- exact.
  2. exp split across TWO engines running concurrently on different
     j-pairs: ACT exp (bf16 out, ~1.03us/unit) and DVE Schraudolph
     (p_bits = round(s*128*log2e + B) as int16 IS bf16 p; ~1.2us/unit;
     the +-3% element error largely cancels in the normalization).
     9/16 pairs on ACT, 7/16 on DVE.  exp bias 0: |s| <= ~77.2 so bf16
     (2^111) and int16 (<=30500) fit; both paths produce e^s.
  3. U matmuls lag ULAG=4 pairs behind scores (tapering to 2 at the
     end) so the in-order PE queue never head-of-line blocks on an exp
     still in flight, and XT may arrive late on the slow SWDGE ring.
  4. j-permutation (query half first) lets the g-projection read the
     same XA2 tiles as scores -- softmax is j-permutation invariant
     (XT is permuted identically).  Each dma_start pays ~2us fixed
     completion latency + ~0.3us/descriptor, so inputs ship as few
     large DMAs with a small first chunk to start the pipeline early.
  5. Batched epilogue per i-macro: ONE strided reciprocal [128,4] and
     ONE broadcast tensor_tensor for all 4 chunks, one 128KB DMA out.

Per-pair steady state: PE ~0.66us (row-tiled concurrent scores pair +
2 U matmuls) vs exp ~1.03-1.2us split over 2 engines.
U matmul packs [y_hi(64) | z_hi | y_lo(62) | z_lo] bf16 rows where
y = x*exp(w2.x_j) and z = exp(w2.x_j) (hi/lo splits for fp32-class
accuracy; z rows give Z); epilogue matmul with WVT2 recombines hi+lo
and extracts Z.
"""
import numpy as np
import ml_dtypes

import concourse.bacc as bacc
import concourse.mybir as mybir
import concourse.tile as tile
from concourse.bass_utils import run_bass_kernel_spmd

F32 = mybir.dt.float32
F32R = mybir.dt.float32r
F16 = mybir.dt.float16
BF16 = mybir.dt.bfloat16
I16 = mybir.dt.int16

B, C, HH, WW = 4, 64, 64, 64
N = HH * WW            # 4096 tokens
NQ = N // 2            # queries per core (2048)
IM = 512               # i-macro size
NIM = NQ // IM         # 4
JT = 128               # j-tile (keys per tile)
NPAIR = N // (2 * JT)  # 16 j-pairs per i-macro
NCH = IM // 128        # output chunks per i-macro (4)
NG = NIM * NPAIR       # 64 global pairs

A_SCALE = float(128.0 * np.log2(np.e))    # Schraudolph slope
SCHRAUD_C = -0.0579                       # mantissa offset (minimax-ish)
B_OFF = float(16256.0 + 128.0 * SCHRAUD_C)
# pairs (of 16 per i-macro) whose exp runs on DVE instead of ACT
DVE_SET = frozenset((3, 4, 5, 9, 10, 11, 15))
ULAG = 4               # U matmuls run this many pairs behind scores

# XA2 chunk column sizes (first chunks small so the pipeline starts fast;
# few chunks overall since each DMA pays ~2us fixed completion latency)
XA_CHUNKS = (512, 1024, 1280, 1280)
XA_OFF = tuple(int(np.sum(XA_CHUNKS[:i])) for i in range(len(XA_CHUNKS)))

_NC_CACHE = {}


def _xa_loc(col):
    """(tile index, column offset) for an absolute XA2 column."""
    for i in range(len(XA_CHUNKS) - 1, -1, -1):
        if col >= XA_OFF[i]:
            return i, col - XA_OFF[i]
    raise ValueError(col)


def build_nc():
    if "nc" in _NC_CACHE:
        return _NC_CACHE["nc"]
    nc = bacc.Bacc(None, target_bir_lowering=False)

    XA2 = nc.dram_tensor("XA2", (128, N), F16, kind="ExternalInput")
    XT = nc.dram_tensor("XT", (128, N // JT, 128), BF16, kind="ExternalInput")
    GW = nc.dram_tensor("GW", (C, 128), F16, kind="ExternalInput")
    WVT2 = nc.dram_tensor("WVT2", (128, C + 2), F32R, kind="ExternalInput")
    OUT = nc.dram_tensor("OUT", (NIM, 128, NCH * C), F32, kind="ExternalOutput")

    with tile.TileContext(nc) as tc:
        with (
            tc.tile_pool(name="consts", bufs=1) as consts,
            tc.tile_pool(name="acts", bufs=1) as acts,
            tc.tile_pool(name="pexp", bufs=7) as pexp,
            tc.tile_pool(name="usbp", bufs=2) as usbp,
            tc.tile_pool(name="rpool", bufs=2) as rpool,
            tc.tile_pool(name="resp", bufs=2) as resp,
            tc.tile_pool(name="psS", bufs=3, space="PSUM") as psS,
            tc.tile_pool(name="psU", bufs=2, space="PSUM") as psU,
        ):
            ebias_sb = consts.tile([128, 1], F32, tag="ebias")
            nc.vector.memset(ebias_sb, 0.0)
            # dummy exp: pulls the ~2.7us ACT table load to the head
            dume_sb = consts.tile([128, 2], F32, tag="dume")
            nc.scalar.activation(dume_sb[:, 0:1], ebias_sb[:, :],
                                 mybir.ActivationFunctionType.Exp)

            gw_sb = consts.tile([C, 128], F16, tag="gw")
            wv2_sb = consts.tile([128, C + 2], F32R, tag="wv2")
            xa2_sb = [consts.tile([128, w], F16, tag=f"xa{t}", name=f"xa{t}")
                      for t, w in enumerate(XA_CHUNKS)]
            xt_sb = consts.tile([128, 32, 128], BF16, tag="xt")
            # rings: scalar = small weights, sync = x chunks (+ the
            # output later), gpsimd SWDGE = the big XT block (only
            # needed once the U matmuls start, ULAG pairs in).
            nc.scalar.dma_start(out=gw_sb, in_=GW[:, :])
            nc.scalar.dma_start(out=wv2_sb, in_=WVT2[:, :])
            for t, w in enumerate(XA_CHUNKS):
                nc.sync.dma_start(out=xa2_sb[t],
                                  in_=XA2[:, XA_OFF[t]:XA_OFF[t] + w])
            nc.gpsimd.dma_start(out=xt_sb, in_=XT[:, :, :])

            # g projection: g = Gm x for the core's 2048 queries (= XA2
            # columns 0-2047 thanks to the j-permutation).  gw holds Gm^T
            # duplicated on both partition halves AND both column halves,
            # so row-tiled matmul pairs project TWO 512-chunks at once
            # and write g to both partition halves (which scores
            # row-tiling needs).  Evac on the Scalar engine (ACT has
            # slack; DVE is the exp engine).
            g_sb = [acts.tile([128, 512], F16, tag=f"g{t}", name=f"g{t}")
                    for t in range(4)]

            def project(sub):
                # one 512-col sub-chunk -> g for i-macro `sub`
                pps = psS.tile([128, 1024], F32, tag="s", name=f"proj{sub}")
                ti, co = _xa_loc(sub * 512)
                nc.tensor.matmul(pps[:, 0:512], gw_sb[:, :],
                                 xa2_sb[ti][0:C, co:co + 512],
                                 start=True, stop=True)
                nc.scalar.activation(g_sb[sub][:, :], pps[:, 0:512],
                                     mybir.ActivationFunctionType.Copy)

            project(0)

            def epilogue(im, u_sb):
                o_ps = psU.tile([128, NCH * (C + 2)], F32, tag="u")
                for ch in range(NCH):
                    nc.tensor.matmul(o_ps[:, ch * 66:ch * 66 + 66],
                                     u_sb[:, ch * 128:(ch + 1) * 128],
                                     wv2_sb[:, :], start=True, stop=True)
                r_sb = rpool.tile([128, NCH], F32, tag="r")
                nc.vector.reciprocal(r_sb[:, :], o_ps[:, C:NCH * 66:66])
                res = resp.tile([128, NCH, C], F32, tag="res")
                o_view = o_ps[:, 0:NCH * 66].rearrange(
                    "p (c f) -> p c f", c=NCH, f=66)[:, :, 0:C]
                r_b = r_sb[:, :, None].broadcast_to([128, NCH, C])
                nc.vector.tensor_tensor(out=res[:, :, :], in0=o_view, in1=r_b,
                                        op=mybir.AluOpType.mult)
                nc.sync.dma_start(
                    out=OUT[im, :, :],
                    in_=res.rearrange("p c f -> p (c f)"))

            # Main loop over 64 global pairs, software-pipelined: the U
            # matmuls for pair g are emitted ULAG pairs later so the
            # in-order PE queue never waits on an exp still in flight.
            p_of = {}
            u_of = {}
            next_u = [0]
            pending = None  # (im, u_sb) epilogue of a finished i-macro

            def u_mms(g):
                im, t = divmod(g, NPAIR)
                jA, jB = 2 * t, 2 * t + 1
                if t == 0:
                    u_of[im] = psU.tile([128, IM], F32, tag="u",
                                        name=f"u{im}")
                u_ps = u_of[im]
                p_sb = p_of.pop(g)
                nc.tensor.matmul(
                    u_ps[:, :], xt_sb[:, jA, :],
                    p_sb[:, 0:512], start=(t == 0), stop=False)
                nc.tensor.matmul(
                    u_ps[:, :], xt_sb[:, jB, :],
                    p_sb[:, 512:1024], start=False, stop=(t == NPAIR - 1))
                if t == NPAIR - 1:
                    u_sb = usbp.tile([128, IM], F32R, tag="u_sb")
                    nc.scalar.activation(u_sb[:, :], u_ps[:, :],
                                         mybir.ActivationFunctionType.Copy)
                    return (im, u_sb)
                return None

            for g in range(NG):
                im, t = divmod(g, NPAIR)
                gh = g_sb[im]
                jA, jB = 2 * t, 2 * t + 1
                tA, cA = _xa_loc(jA * JT)
                tB, cB = _xa_loc(jB * JT)
                s_ps = psS.tile([128, 1024], F32, tag="s")
                nc.tensor.matmul(
                    s_ps[:, 0:512],
                    xa2_sb[tA][0:C, cA:cA + JT],
                    gh[0:C, :],
                    start=True, stop=True, tile_position=(0, 0))
                nc.tensor.matmul(
                    s_ps[:, 512:1024],
                    xa2_sb[tB][C:128, cB:cB + JT],
                    gh[C:128, :],
                    start=True, stop=True, tile_position=(64, 0))
                p_sb = pexp.tile([128, 1024], BF16, tag="p")
                if t in DVE_SET:
                    nc.vector.tensor_scalar(
                        out=p_sb[:, :].bitcast(I16), in0=s_ps[:, :],
                        scalar1=A_SCALE, scalar2=B_OFF,
                        op0=mybir.AluOpType.mult,
                        op1=mybir.AluOpType.add)
                else:
                    nc.scalar.activation(p_sb[:, :], s_ps[:, :],
                                         mybir.ActivationFunctionType.Exp,
                                         bias=ebias_sb[:, :])
                p_of[g] = p_sb
                # taper the U lag 4 -> 2 over the last pairs so the final
                # U matmuls barely trail the last exp
                lag = 2 if g >= NG - 4 else ULAG
                while next_u[0] <= g - lag:
                    fin = u_mms(next_u[0])
                    next_u[0] += 1
                    if fin is not None:
                        pending = fin

                if t == 6 and pending is not None:
                    epilogue(*pending)
                    pending = None
                if g == 2:
                    project(1)
                    project(2)
                if g == 8:
                    project(3)
            while next_u[0] < NG:
                fin = u_mms(next_u[0])
                next_u[0] += 1
                if fin is not None:
                    pending = fin
            epilogue(*pending)
    nc.finalize()
    _NC_CACHE["nc"] = nc
    return nc


def prep_inputs(x, Wq, bq, Wk, bk, Wv, bv):
    """Build the 8 per-core input maps (host-side numpy, cheap)."""
    f32 = np.float32
    f64 = np.float64
    # G-trick: scores s[j, i] = x_j . (Gm x_i) + w2 . x_j with
    # Gm = Wk^T Wq, w2 = Wk^T bq (bk and i-only terms cancel in softmax).
    # The w2 term is folded into XT as a per-j scale exp(w2 . x_j).
    Gm = (Wk.astype(f64).T @ Wq.astype(f64))
    w2 = (Wk.astype(f64).T @ bq.astype(f64))
    gm16 = Gm.T.astype(np.float16)                   # [k, c] = Gm^T
    gw = np.zeros((C, 128), dtype=np.float16)
    gw[:, 0:C] = gm16
    gw[:, C:] = gm16

    # epilogue weights: rows 0-63 Wv^T (for y_hi); rows 64 and 127
    # [bv | 1] (bias + Z from z_hi and z_lo); rows 65-126 Wv^T rows 0-61
    # (for the packed y_lo partials)
    wvt2 = np.zeros((128, C + 2), dtype=f32)
    wvt2[:C, :C] = Wv.T
    wvt2[C, :C] = bv
    wvt2[C, C] = 1.0
    wvt2[C + 1:127, :C] = Wv.T[:C - 2, :]
    wvt2[127, :C] = bv
    wvt2[127, C] = 1.0

    in_maps = []
    for core in range(8):
        b, h = core // 2, core % 2
        xb = np.ascontiguousarray(x[b].reshape(C, N)).astype(f64)
        # j-permutation: the core's own query half first (softmax is
        # permutation-invariant in j; XT uses the same order)
        perm = np.r_[h * NQ:(h + 1) * NQ, (1 - h) * NQ:(2 - h) * NQ]
        xp = xb[:, perm]
        xa2 = np.concatenate([xp, xp], axis=0).astype(np.float16)
        # XT[p, jt, :] = [y_hi(64) | z_hi | y_lo(62) | z_lo] at token
        # jt*128+p, where y = x * exp(w2.x_j), z = exp(w2.x_j)
        zj = np.exp(w2 @ xp)                         # [N]
        y = xp * zj[None, :]
        y_hi = y.astype(ml_dtypes.bfloat16)
        y_lo = (y - y_hi.astype(f64)).astype(ml_dtypes.bfloat16)
        z_hi = zj.astype(ml_dtypes.bfloat16)
        z_lo = (zj - z_hi.astype(f64)).astype(ml_dtypes.bfloat16)
        xt_full = np.zeros((128, N), dtype=ml_dtypes.bfloat16)
        xt_full[:C] = y_hi
        xt_full[C] = z_hi
        xt_full[C + 1:127] = y_lo[:C - 2]
        xt_full[127] = z_lo
        xt = np.ascontiguousarray(
            xt_full.T.reshape(N // JT, 128, 128).transpose(1, 0, 2))
        in_maps.append(dict(XA2=xa2, XT=xt, GW=gw, WVT2=wvt2))
    return in_maps


def assemble_output(results):
    out = np.empty((B, C, N), dtype=np.float32)
    for core in range(8):
        b, h = core // 2, core % 2
        o = results[core]["OUT"]                        # [NIM, 128, NCH*C]
        o = o.reshape(NIM, 128, NCH, C).transpose(0, 2, 1, 3).reshape(NQ, C)
        out[b, :, h * NQ:(h + 1) * NQ] = o.T
    return out.reshape(B, C, HH, WW)


def kernel(x, Wq, bq, Wk, bk, Wv, bv, **run_kwargs):
    x = np.asarray(x, dtype=np.float32)
    nc = build_nc()
    in_maps = prep_inputs(np.asarray(x), np.asarray(Wq), np.asarray(bq),
                          np.asarray(Wk), np.asarray(bk),
                          np.asarray(Wv), np.asarray(bv))
    res = run_bass_kernel_spmd(nc, in_maps, core_ids=list(range(8)),
                               **run_kwargs)
    out = assemble_output(res.results)
    if run_kwargs:
        return out, res
    return out


if __name__ == "__main__":
    rng = np.random.default_rng(0)
    s = 1.0 / np.sqrt(C)
    x = rng.standard_normal((B, C, HH, WW), dtype=np.float32)
    args = dict(
        x=x,
        Wq=(rng.standard_normal((C, C), dtype=np.float32) * s),
        bq=(rng.standard_normal(C, dtype=np.float32) * 0.01),
        Wk=(rng.standard_normal((C, C), dtype=np.float32) * s),
        bk=(rng.standard_normal(C, dtype=np.float32) * 0.01),
        Wv=(rng.standard_normal((C, C), dtype=np.float32) * s),
        bv=(rng.standard_normal(C, dtype=np.float32) * 0.01),
    )
    out = kernel(**args)
    print("kernel output:", out.shape, out.dtype)
